# revision 5
# baseline (speedup 1.0000x reference)
"""GRU (CustomRNN) Trainium2 kernel.

Strategy: data-parallel over batch (256 -> 8 cores x 32). Per core:
  - x pre-transposed on host to [4, 128, SEQ*32] (d_in on partitions,
    column = t*32 + b); one zero-padded extra block so the software
    pipeline can prefetch past the end.
  - Recurrent state h kept ONLY in bf16, packed-T [128, 128]
    (partition q, column 32k+b) representing hT[i=128k+q, b].
  - Per 64-step block: x@W_{z,r,h} + bias precomputed into bf16 A
    tiles in SBUF; the A matmuls for block ib+1 are INTERLEAVED through
    the recurrent steps of block ib (one 4-MM chunk + writeback per
    step) so the PE never sees an un-hidden A burst.
  - Per step: ONE identity matmul injects A[t] (all 3 gates, [128,384])
    into a single PSUM tile with start=True; the 48 U matmuls (bf16
    stationary, 32-col moving hT slices) accumulate on top, k-outer.
    Sub-tile dep tracking lets sigma(r) fire as soon as the r region is
    done while z/htild MMs are still streaming.
  - Update: h' = (1-z)*h + z*htild with 1-z = sigmoid(-z_pre) on the
    scalar engine, t1 = (1-z)*h on gpsimd during the tanh window; t2
    and the final add are split into column halves so the next step's
    r matmuls (k-outer) can start after the low half of h' lands.
  - After all steps: relu(h) @ Wfc (bf16) on device; gather + bias +
    batch-axis log_softmax on host (softmax crosses cores).
"""

import numpy as np

import concourse.bass as bass
import concourse.mybir as mybir
import concourse.tile as tile
from concourse import bacc
from concourse.bass import ds
from concourse.bass_utils import run_bass_kernel_spmd

SEQ, BATCH, D_IN, D_HID, D_OUT = 2048, 256, 512, 512, 1000
NCORES = 8
BS = BATCH // NCORES          # 32 batch rows per core
KI = D_IN // 128              # 4 contraction chunks for x@W
KH = D_HID // 128             # 4 contraction chunks for h@U
T_BLK = 64                    # timesteps per block
F32 = mybir.dt.float32
BF16 = mybir.dt.bfloat16
AF = mybir.ActivationFunctionType


def build_bass(seq=SEQ, t_blk=T_BLK):
    assert seq % t_blk == 0
    nblk = seq // t_blk
    assert nblk % 2 == 0
    CB = t_blk * BS           # x/A columns per block
    NCH = 512                 # psum free-dim chunk for the A matmuls
    nch = CB // NCH
    tch = NCH // BS           # timesteps per A psum chunk
    G = 3 * D_HID
    HB = KH * BS              # 128: packed-T h columns
    H2 = HB // 2
    A_START = t_blk - 3 * nch * KH  # steps carrying interleaved A jobs

    nc = bacc.Bacc(None, target_bir_lowering=False)

    # one extra (zero) block of x for the software-pipeline prefetch
    x_d = nc.dram_tensor("xt", [KI, 128, (seq + t_blk) * BS], BF16,
                         kind="ExternalInput")
    w_d = nc.dram_tensor("w", [KI, 128, G], BF16, kind="ExternalInput")
    u_d = nc.dram_tensor("u", [KH, 128, G], BF16, kind="ExternalInput")
    b_d = nc.dram_tensor("bias", [128, 12], F32, kind="ExternalInput")
    i_d = nc.dram_tensor("ident", [128, 128], BF16, kind="ExternalInput")
    wfc_d = nc.dram_tensor("wfc", [KH, 128, D_OUT], BF16, kind="ExternalInput")
    out_d = nc.dram_tensor("out", [BS, D_OUT], F32, kind="ExternalOutput")

    with tile.TileContext(nc) as tc:
        with (
            tc.tile_pool(name="const", bufs=1) as constp,
            tc.tile_pool(name="st", bufs=2) as stp,
            tc.tile_pool(name="pg", bufs=4, space="PSUM") as psp,
            tc.tile_pool(name="psA", bufs=2, space="PSUM") as psa,
        ):
            u_sb = constp.tile([128, KH, G], BF16)
            w_sb = constp.tile([128, KI, G], BF16)
            b_sb = constp.tile([128, 12], F32)
            ident = constp.tile([128, 128], BF16)
            for k in range(KH):
                nc.sync.dma_start(u_sb[:, k, :], u_d[k])
            for k in range(KI):
                nc.sync.dma_start(w_sb[:, k, :], w_d[k])
            nc.sync.dma_start(b_sb[:], b_d[:])
            nc.sync.dma_start(ident[:], i_d[:])

            # double-buffered x block + A block (bf16)
            xblk = [constp.tile([128, KI, CB], BF16, name=f"xblk{i}")
                    for i in range(2)]
            a_sb = [constp.tile([128, t_blk, 3, HB], BF16, name=f"a_sb{i}")
                    for i in range(2)]

            # ping/pong recurrent state, packed-T [128, 128] bf16 only
            hb = [constp.tile([128, HB], BF16, name=f"hb{i}")
                  for i in range(2)]
            nc.vector.memset(hb[0][:], 0.0)

            def emit_a_chunk(buf, g, mj, ci):
                """x@W matmuls + bias writeback for one A chunk."""
                w_tile = w_sb[:, :, g * D_HID + mj * 128:
                              g * D_HID + (mj + 1) * 128]
                pa = psa.tile([128, NCH], F32, tag="pa")
                for k in range(KI):
                    nc.tensor.matmul(
                        pa[:],
                        w_tile[:, k, :],
                        xblk[buf][:, k, ci * NCH:(ci + 1) * NCH],
                        start=(k == 0),
                        stop=(k == KI - 1),
                    )
                t0 = ci * tch
                a_out = a_sb[buf][:, t0:t0 + tch, g, mj * BS:(mj + 1) * BS]
                bias_ap = b_sb[:, g * 4 + mj:g * 4 + mj + 1]
                nc.vector.tensor_add(
                    a_out,
                    pa[:].rearrange("p (t b) -> p t b", b=BS),
                    bias_ap[:, :, None].to_broadcast((128, tch, BS)),
                )

            def emit_ident(buf, t):
                """Allocate step-psum and inject A[t] (all 3 gates)."""
                pg = psp.tile([128, 3 * HB], F32, tag="pg")
                nc.tensor.matmul(
                    pg[:], ident[:],
                    a_sb[buf][:, t, :, :].rearrange("p g c -> p (g c)"),
                    start=True, stop=False, skip_group_check=True)
                return pg

            def step(t, pg, ident_src, a_job):
                """Emit one recurrent step; returns psum tile for t+1."""
                hin = hb[t % 2]
                hout = hb[(t + 1) % 2]

                zoff, roff, hoff = 0, HB, 2 * HB      # pg column regions
                uz, ur, uh = 0, D_HID, 2 * D_HID      # u_sb column offsets

                # r gate first (it gates the htild matmul), then z; k-outer
                for gu, goff in ((ur, roff), (uz, zoff)):
                    for k in range(KH):
                        for mj in range(KH):
                            nc.tensor.matmul(
                                pg[:, goff + mj * BS:goff + (mj + 1) * BS],
                                u_sb[:, k, gu + mj * 128:gu + (mj + 1) * 128],
                                hin[:, k * BS:(k + 1) * BS],
                                start=False,
                                stop=(k == KH - 1),
                                skip_group_check=True,
                            )

                r_act = stp.tile([128, HB], BF16, tag="r_act")
                nc.scalar.activation(r_act[:], pg[:, roff:roff + HB],
                                     AF.Sigmoid)
                rh = stp.tile([128, HB], BF16, tag="rh")
                nc.vector.tensor_mul(rh[:], r_act[:], hin[:])

                for k in range(KH):
                    for mj in range(KH):
                        nc.tensor.matmul(
                            pg[:, hoff + mj * BS:hoff + (mj + 1) * BS],
                            u_sb[:, k, uh + mj * 128:uh + (mj + 1) * 128],
                            rh[:, k * BS:(k + 1) * BS],
                            start=False,
                            stop=(k == KH - 1),
                            skip_group_check=True,
                        )

                # fill PE idle during tanh/update: next step's A inject +
                # one interleaved A chunk for the next block
                pg_next = emit_ident(*ident_src)
                if a_job is not None:
                    emit_a_chunk(*a_job)

                z_act = stp.tile([128, HB], BF16, tag="z_act")
                nc.scalar.activation(z_act[:], pg[:, zoff:zoff + HB],
                                     AF.Sigmoid)
                zc_act = stp.tile([128, HB], BF16, tag="zc_act")
                nc.scalar.activation(zc_act[:], pg[:, zoff:zoff + HB],
                                     AF.Sigmoid, scale=-1.0)
                # t1 = (1-z)*h on gpsimd, ready before tanh completes
                t1 = stp.tile([128, HB], BF16, tag="t1")
                nc.gpsimd.tensor_mul(t1[:], zc_act[:], hin[:])

                ht = stp.tile([128, HB], BF16, tag="ht")
                nc.scalar.activation(ht[:], pg[:, hoff:hoff + HB], AF.Tanh)
                t2 = stp.tile([128, HB], BF16, tag="t2")
                # halves: the low half of h' unblocks next step's k=0,1
                # r matmuls while the high half is still in flight
                nc.vector.tensor_mul(t2[:, :H2], z_act[:, :H2], ht[:, :H2])
                nc.vector.tensor_add(hout[:, :H2], t1[:, :H2], t2[:, :H2])
                nc.vector.tensor_mul(t2[:, H2:], z_act[:, H2:], ht[:, H2:])
                nc.vector.tensor_add(hout[:, H2:], t1[:, H2:], t2[:, H2:])
                return pg_next

            def a_jobs(buf):
                """One A chunk per step from A_START on (48 jobs)."""
                jobs = [None] * A_START + [
                    (buf, g, mj, ci)
                    for ci in range(nch)
                    for g in range(3)
                    for mj in range(KH)]
                assert len(jobs) == t_blk
                return jobs

            # ---- prologue: block 0 DMA + A burst + first ident
            nc.sync.dma_start(
                xblk[0][:],
                x_d[:, :, ds(0, CB)].rearrange("k q c -> q k c"))
            for ci in range(nch):
                for g in range(3):
                    for mj in range(KH):
                        emit_a_chunk(0, g, mj, ci)
            pg = emit_ident(0, 0)

            # ---- main loop: 2 blocks per HW iteration (buffer parity)
            with tc.For_i(0, nblk // 2, 1,
                          hint_engines=(mybir.EngineType.PE,)) as ib:
                for par in range(2):
                    cur, nxt = par % 2, (par + 1) % 2
                    # prefetch x for the next block
                    nc.sync.dma_start(
                        xblk[nxt][:],
                        x_d[:, :, ds(ib * 2 * CB + (par + 1) * CB, CB)]
                        .rearrange("k q c -> q k c"))
                    jobs = a_jobs(nxt)
                    for t in range(t_blk):
                        ident_src = ((cur, t + 1) if t < t_blk - 1
                                     else (nxt, 0))
                        pg = step(t, pg, ident_src, jobs[t])

            # final state lands in hb[0]; fc head
            wfc_sb = constp.tile([128, KH, D_OUT], BF16)
            for k in range(KH):
                nc.sync.dma_start(wfc_sb[:, k, :], wfc_d[k])
            hrelu = stp.tile([128, HB], BF16, tag="hrelu")
            nc.scalar.activation(hrelu[:], hb[0][:], AF.Relu)
            out_sb = stp.tile([BS, D_OUT], F32, tag="outsb")
            for ci in range(2):
                n0, nsz = ci * 500, 500
                po = psa.tile([128, NCH], F32, tag="pa")
                for k in range(KH):
                    nc.tensor.matmul(
                        po[:BS, :nsz],
                        hrelu[:, k * BS:(k + 1) * BS],
                        wfc_sb[:, k, n0:n0 + nsz],
                        start=(k == 0),
                        stop=(k == KH - 1),
                    )
                nc.vector.tensor_copy(out_sb[:, n0:n0 + nsz], po[:BS, :nsz])
            nc.sync.dma_start(out_d[:], out_sb[:])

    nc.finalize()
    return nc


def _prep_inputs(x, Wz, Uz, Wr, Ur, Wh, Uh, bz, buz, br, bur, bh, buh, Wfc,
                 t_blk=T_BLK):
    import ml_dtypes
    seq = x.shape[0]
    # x[t, 32c+b, 128k+q] -> xt[c][k, q, t*32+b], one zero pad block
    xr = x.reshape(seq, NCORES, BS, KI, 128).transpose(1, 3, 4, 0, 2)
    xt = np.zeros((NCORES, KI, 128, (seq + t_blk) * BS), ml_dtypes.bfloat16)
    xt[:, :, :, :seq * BS] = xr.astype(ml_dtypes.bfloat16).reshape(
        NCORES, KI, 128, seq * BS)

    w_all = np.concatenate([Wz, Wr, Wh], axis=1)     # [512, 1536]
    u_all = np.concatenate([Uz, Ur, Uh], axis=1)
    w_dev = np.ascontiguousarray(
        w_all.reshape(KI, 128, 3 * D_HID)).astype(ml_dtypes.bfloat16)
    u_dev = np.ascontiguousarray(
        u_all.reshape(KH, 128, 3 * D_HID)).astype(ml_dtypes.bfloat16)
    b_all = np.stack([bz + buz, br + bur, bh + buh])  # [3, 512]
    b_dev = np.ascontiguousarray(
        b_all.reshape(3, 4, 128).transpose(2, 0, 1).reshape(128, 12))
    i_dev = np.eye(128, dtype=np.float32).astype(ml_dtypes.bfloat16)
    wfc_dev = np.ascontiguousarray(
        Wfc.reshape(KH, 128, D_OUT)).astype(ml_dtypes.bfloat16)
    return xt, w_dev, u_dev, b_dev, i_dev, wfc_dev


def make_in_maps(inputs, seq=SEQ, t_blk=T_BLK):
    f = lambda k: np.ascontiguousarray(np.asarray(inputs[k], dtype=np.float32))
    x = f("x")[:seq]
    xt, w_dev, u_dev, b_dev, i_dev, wfc_dev = _prep_inputs(
        x, f("Wz"), f("Uz"), f("Wr"), f("Ur"), f("Wh"), f("Uh"),
        f("bz"), f("buz"), f("br"), f("bur"), f("bh"), f("buh"), f("Wfc"),
        t_blk=t_blk)
    return [
        {"xt": xt[c], "w": w_dev, "u": u_dev, "bias": b_dev, "ident": i_dev,
         "wfc": wfc_dev}
        for c in range(NCORES)
    ]


def run_gru(inputs, seq=SEQ, t_blk=T_BLK, trace=False):
    in_maps = make_in_maps(inputs, seq=seq, t_blk=t_blk)
    nc = build_bass(seq=seq, t_blk=t_blk)
    res = run_bass_kernel_spmd(nc, in_maps, core_ids=list(range(NCORES)),
                               trace=trace)
    logits = np.concatenate([res.results[c]["out"] for c in range(NCORES)], 0)
    logits = logits + np.asarray(inputs["bfc"], np.float32)[None, :]
    m = logits.max(axis=0, keepdims=True)
    lse = m + np.log(np.exp(logits - m).sum(axis=0, keepdims=True))
    out = (logits - lse)[None]
    return out.astype(np.float32), res


def kernel(**inputs) -> np.ndarray:
    out, _ = run_gru(inputs, seq=SEQ, t_blk=T_BLK)
    return out


# revision 6
# speedup vs baseline: 1.3300x; 1.3300x over previous
"""GRU (CustomRNN) Trainium2 kernel.

Strategy: data-parallel over batch (256 -> 8 cores x 32). Per core:
  - x pre-transposed on host to [4, 128, SEQ*32] (d_in on partitions,
    column = t*32 + b); one zero-padded extra block so the software
    pipeline can prefetch past the end.
  - Recurrent state h kept ONLY in bf16, packed-T, split into two
    [128, 64] tiles (hidden chunks 0-1 / 2-3) so the low half of the
    update unblocks the next step's k=0,1 matmuls early.
  - Per 64-step block: x@W_{z,r,h} + bias precomputed into bf16 A
    tiles in SBUF; the A matmuls for block ib+1 are INTERLEAVED through
    the recurrent steps of block ib (one 4-MM chunk + writeback per
    step) so the PE never sees an un-hidden A burst.
  - Per step: 3 identity matmuls inject A[t] into per-gate PSUM tiles
    (start=True, A-dependent only, so the scheduler hoists them into
    PE idle); 48 U matmuls (bf16, 32-col moving hT slices) accumulate
    on top, k-outer so partial h halves feed them.
  - Update: h' = (1-z)*h + z*htild; zc = 1-z on vector, t1 = zc*h on
    gpsimd (off the critical path), t2/add split lo/hi halves.
  - After all steps: relu(h) @ Wfc (bf16) on device; gather + bias +
    batch-axis log_softmax on host (softmax crosses cores).
"""

import numpy as np

import concourse.bass as bass
import concourse.mybir as mybir
import concourse.tile as tile
from concourse import bacc
from concourse.bass import ds
from concourse.bass_utils import run_bass_kernel_spmd

SEQ, BATCH, D_IN, D_HID, D_OUT = 2048, 256, 512, 512, 1000
NCORES = 8
BS = BATCH // NCORES          # 32 batch rows per core
KI = D_IN // 128              # 4 contraction chunks for x@W
KH = D_HID // 128             # 4 contraction chunks for h@U
T_BLK = 64                    # timesteps per block
F32 = mybir.dt.float32
BF16 = mybir.dt.bfloat16
AF = mybir.ActivationFunctionType
ALU = mybir.AluOpType


def build_bass(seq=SEQ, t_blk=T_BLK):
    assert seq % t_blk == 0
    nblk = seq // t_blk
    assert nblk % 2 == 0
    CB = t_blk * BS           # x/A columns per block
    NCH = 512                 # psum free-dim chunk for the A matmuls
    nch = CB // NCH
    tch = NCH // BS           # timesteps per A psum chunk
    G = 3 * D_HID
    HB = KH * BS              # 128: packed-T h columns
    H2 = HB // 2

    nc = bacc.Bacc(None, target_bir_lowering=False)

    # one extra (zero) block of x for the software-pipeline prefetch
    x_d = nc.dram_tensor("xt", [KI, 128, (seq + t_blk) * BS], BF16,
                         kind="ExternalInput")
    w_d = nc.dram_tensor("w", [KI, 128, G], BF16, kind="ExternalInput")
    u_d = nc.dram_tensor("u", [KH, 128, G], BF16, kind="ExternalInput")
    b_d = nc.dram_tensor("bias", [128, 12], F32, kind="ExternalInput")
    i_d = nc.dram_tensor("ident", [128, 128], BF16, kind="ExternalInput")
    wfc_d = nc.dram_tensor("wfc", [KH, 128, D_OUT], BF16, kind="ExternalInput")
    out_d = nc.dram_tensor("out", [BS, D_OUT], F32, kind="ExternalOutput")

    with tile.TileContext(nc) as tc:
        with (
            tc.tile_pool(name="const", bufs=1) as constp,
            tc.tile_pool(name="st", bufs=2) as stp,
            tc.tile_pool(name="ps", bufs=2, space="PSUM") as psp,
            tc.tile_pool(name="psA", bufs=2, space="PSUM") as psa,
        ):
            u_sb = constp.tile([128, KH, G], BF16)
            w_sb = constp.tile([128, KI, G], BF16)
            b_sb = constp.tile([128, 12], F32)
            ident = constp.tile([128, 128], BF16)
            for k in range(KH):
                nc.sync.dma_start(u_sb[:, k, :], u_d[k])
            for k in range(KI):
                nc.sync.dma_start(w_sb[:, k, :], w_d[k])
            nc.sync.dma_start(b_sb[:], b_d[:])
            nc.sync.dma_start(ident[:], i_d[:])

            # double-buffered x block + A block (bf16)
            xblk = [constp.tile([128, KI, CB], BF16, name=f"xblk{i}")
                    for i in range(2)]
            a_sb = [constp.tile([128, t_blk, 3, HB], BF16, name=f"a_sb{i}")
                    for i in range(2)]

            # ping/pong recurrent state, packed-T bf16, split lo/hi
            hlo = [constp.tile([128, H2], BF16, name=f"hlo{i}")
                   for i in range(2)]
            hhi = [constp.tile([128, H2], BF16, name=f"hhi{i}")
                   for i in range(2)]
            nc.vector.memset(hlo[0][:], 0.0)
            nc.vector.memset(hhi[0][:], 0.0)

            def hsrc(tiles, k):
                """[128,32] moving slice for contraction chunk k."""
                return tiles[k // 2][:, (k % 2) * BS:(k % 2 + 1) * BS]

            def emit_a_chunk(buf, g, mj, ci):
                """x@W matmuls + bias writeback for one A chunk."""
                w_tile = w_sb[:, :, g * D_HID + mj * 128:
                              g * D_HID + (mj + 1) * 128]
                pa = psa.tile([128, NCH], F32, tag="pa")
                for k in range(KI):
                    nc.tensor.matmul(
                        pa[:],
                        w_tile[:, k, :],
                        xblk[buf][:, k, ci * NCH:(ci + 1) * NCH],
                        start=(k == 0),
                        stop=(k == KI - 1),
                    )
                t0 = ci * tch
                a_out = a_sb[buf][:, t0:t0 + tch, g, mj * BS:(mj + 1) * BS]
                bias_ap = b_sb[:, g * 4 + mj:g * 4 + mj + 1]
                nc.vector.tensor_add(
                    a_out,
                    pa[:].rearrange("p (t b) -> p t b", b=BS),
                    bias_ap[:, :, None].to_broadcast((128, tch, BS)),
                )

            def step(buf, t, a_job):
                hin = (hlo[t % 2], hhi[t % 2])
                hout = (hlo[(t + 1) % 2], hhi[(t + 1) % 2])

                pr = psp.tile([128, HB], F32, tag="pr")
                pz = psp.tile([128, HB], F32, tag="pz")
                ph = psp.tile([128, HB], F32, tag="ph")
                # inject A into psum (A-dependent only, hoists early)
                nc.tensor.matmul(pr[:], ident[:], a_sb[buf][:, t, 1, :],
                                 start=True, stop=False,
                                 skip_group_check=True)
                nc.tensor.matmul(pz[:], ident[:], a_sb[buf][:, t, 0, :],
                                 start=True, stop=False,
                                 skip_group_check=True)
                nc.tensor.matmul(ph[:], ident[:], a_sb[buf][:, t, 2, :],
                                 start=True, stop=False,
                                 skip_group_check=True)

                # r gate first (it gates htild), then z; k-outer
                for ps, gu in ((pr, D_HID), (pz, 0)):
                    for k in range(KH):
                        for mj in range(KH):
                            nc.tensor.matmul(
                                ps[:, mj * BS:(mj + 1) * BS],
                                u_sb[:, k, gu + mj * 128:gu + (mj + 1) * 128],
                                hsrc(hin, k),
                                start=False,
                                stop=(k == KH - 1),
                                skip_group_check=True,
                            )

                r_act = stp.tile([128, HB], BF16, tag="r_act")
                nc.scalar.activation(r_act[:], pr[:], AF.Sigmoid)
                rlo = stp.tile([128, H2], BF16, tag="rlo")
                rhi = stp.tile([128, H2], BF16, tag="rhi")
                nc.vector.tensor_mul(rlo[:], r_act[:, :H2], hin[0][:])
                nc.vector.tensor_mul(rhi[:], r_act[:, H2:], hin[1][:])

                uh = 2 * D_HID
                for k in range(KH):
                    for mj in range(KH):
                        nc.tensor.matmul(
                            ph[:, mj * BS:(mj + 1) * BS],
                            u_sb[:, k, uh + mj * 128:uh + (mj + 1) * 128],
                            hsrc((rlo, rhi), k),
                            start=False,
                            stop=(k == KH - 1),
                            skip_group_check=True,
                        )

                # one interleaved A chunk for the next block
                if a_job is not None:
                    emit_a_chunk(*a_job)

                z_act = stp.tile([128, HB], BF16, tag="z_act")
                nc.scalar.activation(z_act[:], pz[:], AF.Sigmoid)
                # zc = 1 - z on vector; t1 = zc*h on gpsimd (off-path)
                zc_act = stp.tile([128, HB], BF16, tag="zc_act")
                nc.vector.tensor_scalar(zc_act[:], z_act[:], -1.0, 1.0,
                                        ALU.mult, ALU.add)
                t1lo = stp.tile([128, H2], BF16, tag="t1lo")
                t1hi = stp.tile([128, H2], BF16, tag="t1hi")
                nc.gpsimd.tensor_mul(t1lo[:], zc_act[:, :H2], hin[0][:])
                nc.gpsimd.tensor_mul(t1hi[:], zc_act[:, H2:], hin[1][:])

                ht = stp.tile([128, HB], BF16, tag="ht")
                nc.scalar.activation(ht[:], ph[:], AF.Tanh)
                t2 = stp.tile([128, HB], BF16, tag="t2")
                nc.vector.tensor_mul(t2[:, :H2], z_act[:, :H2], ht[:, :H2])
                nc.vector.tensor_add(hout[0][:], t1lo[:], t2[:, :H2])
                nc.vector.tensor_mul(t2[:, H2:], z_act[:, H2:], ht[:, H2:])
                nc.vector.tensor_add(hout[1][:], t1hi[:], t2[:, H2:])

            def a_jobs(buf):
                """A-chunk job list spread over the steps of a block."""
                jobs = [(buf, g, mj, ci)
                        for ci in range(nch)
                        for g in range(3)
                        for mj in range(KH)]
                start = max(4, t_blk - len(jobs))
                per_step = [None] * t_blk
                slots = t_blk - start
                assert slots >= len(jobs), (t_blk, len(jobs))
                for i, job in enumerate(jobs):
                    per_step[start + i] = job
                return per_step

            # ---- prologue: block 0 DMA + A burst
            nc.sync.dma_start(
                xblk[0][:],
                x_d[:, :, ds(0, CB)].rearrange("k q c -> q k c"))
            for ci in range(nch):
                for g in range(3):
                    for mj in range(KH):
                        emit_a_chunk(0, g, mj, ci)

            # ---- main loop: 2 blocks per HW iteration (buffer parity)
            with tc.For_i(0, nblk // 2, 1,
                          hint_engines=(mybir.EngineType.PE,)) as ib:
                for par in range(2):
                    cur, nxt = par % 2, (par + 1) % 2
                    # prefetch x for the next block
                    nc.sync.dma_start(
                        xblk[nxt][:],
                        x_d[:, :, ds(ib * 2 * CB + (par + 1) * CB, CB)]
                        .rearrange("k q c -> q k c"))
                    jobs = a_jobs(nxt)
                    for t in range(t_blk):
                        step(cur, t, jobs[t])

            # final state lands in (hlo/hhi)[0]; fc head
            wfc_sb = constp.tile([128, KH, D_OUT], BF16)
            for k in range(KH):
                nc.sync.dma_start(wfc_sb[:, k, :], wfc_d[k])
            hrelu = stp.tile([128, HB], BF16, tag="hrelu")
            nc.scalar.activation(hrelu[:, :H2], hlo[0][:], AF.Relu)
            nc.scalar.activation(hrelu[:, H2:], hhi[0][:], AF.Relu)
            out_sb = stp.tile([BS, D_OUT], F32, tag="outsb")
            for ci in range(2):
                n0, nsz = ci * 500, 500
                po = psa.tile([128, NCH], F32, tag="pa")
                for k in range(KH):
                    nc.tensor.matmul(
                        po[:BS, :nsz],
                        hrelu[:, k * BS:(k + 1) * BS],
                        wfc_sb[:, k, n0:n0 + nsz],
                        start=(k == 0),
                        stop=(k == KH - 1),
                    )
                nc.vector.tensor_copy(out_sb[:, n0:n0 + nsz], po[:BS, :nsz])
            nc.sync.dma_start(out_d[:], out_sb[:])

    nc.finalize()
    return nc


def _prep_inputs(x, Wz, Uz, Wr, Ur, Wh, Uh, bz, buz, br, bur, bh, buh, Wfc,
                 t_blk=T_BLK):
    import ml_dtypes
    seq = x.shape[0]
    # x[t, 32c+b, 128k+q] -> xt[c][k, q, t*32+b], one zero pad block
    xr = x.reshape(seq, NCORES, BS, KI, 128).transpose(1, 3, 4, 0, 2)
    xt = np.zeros((NCORES, KI, 128, (seq + t_blk) * BS), ml_dtypes.bfloat16)
    xt[:, :, :, :seq * BS] = xr.astype(ml_dtypes.bfloat16).reshape(
        NCORES, KI, 128, seq * BS)

    w_all = np.concatenate([Wz, Wr, Wh], axis=1)     # [512, 1536]
    u_all = np.concatenate([Uz, Ur, Uh], axis=1)
    w_dev = np.ascontiguousarray(
        w_all.reshape(KI, 128, 3 * D_HID)).astype(ml_dtypes.bfloat16)
    u_dev = np.ascontiguousarray(
        u_all.reshape(KH, 128, 3 * D_HID)).astype(ml_dtypes.bfloat16)
    b_all = np.stack([bz + buz, br + bur, bh + buh])  # [3, 512]
    b_dev = np.ascontiguousarray(
        b_all.reshape(3, 4, 128).transpose(2, 0, 1).reshape(128, 12))
    i_dev = np.eye(128, dtype=np.float32).astype(ml_dtypes.bfloat16)
    wfc_dev = np.ascontiguousarray(
        Wfc.reshape(KH, 128, D_OUT)).astype(ml_dtypes.bfloat16)
    return xt, w_dev, u_dev, b_dev, i_dev, wfc_dev


def make_in_maps(inputs, seq=SEQ, t_blk=T_BLK):
    f = lambda k: np.ascontiguousarray(np.asarray(inputs[k], dtype=np.float32))
    x = f("x")[:seq]
    xt, w_dev, u_dev, b_dev, i_dev, wfc_dev = _prep_inputs(
        x, f("Wz"), f("Uz"), f("Wr"), f("Ur"), f("Wh"), f("Uh"),
        f("bz"), f("buz"), f("br"), f("bur"), f("bh"), f("buh"), f("Wfc"),
        t_blk=t_blk)
    return [
        {"xt": xt[c], "w": w_dev, "u": u_dev, "bias": b_dev, "ident": i_dev,
         "wfc": wfc_dev}
        for c in range(NCORES)
    ]


def run_gru(inputs, seq=SEQ, t_blk=T_BLK, trace=False):
    in_maps = make_in_maps(inputs, seq=seq, t_blk=t_blk)
    nc = build_bass(seq=seq, t_blk=t_blk)
    res = run_bass_kernel_spmd(nc, in_maps, core_ids=list(range(NCORES)),
                               trace=trace)
    logits = np.concatenate([res.results[c]["out"] for c in range(NCORES)], 0)
    logits = logits + np.asarray(inputs["bfc"], np.float32)[None, :]
    m = logits.max(axis=0, keepdims=True)
    lse = m + np.log(np.exp(logits - m).sum(axis=0, keepdims=True))
    out = (logits - lse)[None]
    return out.astype(np.float32), res


def kernel(**inputs) -> np.ndarray:
    out, _ = run_gru(inputs, seq=SEQ, t_blk=T_BLK)
    return out
